# revision 49
# baseline (speedup 1.0000x reference)
"""CAMIL self-attention kernel for 8 Trainium2 NeuronCores.

Reference computation (per bag b of B=4, N=4096 instances, D=512 features):
    qk = x @ W_qk.T ; q, k = split(qk)          (att dim E=64)
    v  = x @ W_v.T
    logits_n = (1/8) * sum_m adj[n,m] * (q_n . k_m)
             = (q_n/8) . (adj @ k)_n
    w = softmax(logits over N) ; out = w * v

Sharding: 2 cores per bag, each core owns NH=2048 rows (query dim). Inputs
are pre-rotated on host so every core's rows are local tiles 0..15; the
softmax normalization is completed with a tiny AllReduce of the local
sumexp between the two cores of each bag.

Numerics: x/W in bf16 (PE multiplies at fp22, accumulates fp32), adj exact
in fp8 e4m3, K split into fp8 hi+lo planes packed side-by-side in the
DoubleRow stationary operand so a single pass over the adjacency computes
both planes into PSUM partitions 0:64 (hi) and 64:128 (lo) at the fp8
DoubleRow rate. The adjacency streams n-block-major (4 column blocks of
512 rows each) so each block's logits, exp and sum-partial complete while
later blocks are still in flight - only the last block's short tail sits
after the final DMA byte. V-projection matmuls are front-loaded into the
early blocks to keep DVE/ACT clear for the late-block tails. Logit dot +
softmax in
fp32 on DVE/ACT; a fixed shift exp(l - 80) keeps exp in fp32 range
(logits ~N(0,20), per-bag max ~75-100) and removes the serial
max-reduction from the critical path.
"""

import sys

sys.path.insert(0, "/opt/trn_rl_repo")

import numpy as np

import concourse.bass as bass
import concourse.tile as tile
from concourse import bacc, bass_isa, mybir
from concourse.bass_utils import run_bass_kernel_spmd
from concourse.masks import make_identity

B, N, D, E = 4, 4096, 512, 64
P = 128
NCORES = 8
NH = N // 2        # rows per core
MT = N // P        # 32 m-tiles per bag
MP = MT // 2       # 16 m-tile pairs (DoubleRow covers 2 k-tiles/inst)
TH = NH // P       # 16 row-tiles per core
DT = D // P        # 4 d-tiles
F32 = mybir.dt.float32
BF16 = mybir.dt.bfloat16
F8 = mybir.dt.float8e4
DR = mybir.MatmulPerfMode.DoubleRow
GROUPS = [[0, 1], [2, 3], [4, 5], [6, 7]]


def _build(single=False):
    # single=True: replace the cross-core AllReduce with a local DMA so the
    # module has no collectives (for TimelineSim cost modeling only).
    nc = bacc.Bacc("TRN2", target_bir_lowering=False, num_devices=NCORES)

    xt = nc.dram_tensor("xt", [D, N], BF16, kind="ExternalInput")
    at = nc.dram_tensor("at", [N, NH], F8, kind="ExternalInput")
    wqkt = nc.dram_tensor("wqkt", [D, 2 * E], BF16, kind="ExternalInput")
    wvt = nc.dram_tensor("wvt", [D, D], BF16, kind="ExternalInput")
    out = nc.dram_tensor("out", [NH, D], F32, kind="ExternalOutput")

    xt_v = xt.ap().rearrange("(o p) n -> p o n", p=P)        # [128, 4, 4096]
    at_v = at.ap().rearrange("(mo p) n -> p mo n", p=P)      # [128, 32, 2048]
    wqkt_v = wqkt.ap().rearrange("(o p) e -> p o e", p=P)    # [128, 4, 128]
    wvt_v = wvt.ap().rearrange("(o p) e -> p o e", p=P)      # [128, 4, 512]
    out_v = out.ap().rearrange("(t p) e -> p t e", p=P)      # [128, 16, 512]

    with tile.TileContext(nc) as tc:
        with tc.tile_pool(name="big", bufs=1) as big, \
             tc.tile_pool(name="atp", bufs=16) as atp, \
             tc.tile_pool(name="ostream", bufs=4) as ostream, \
             tc.tile_pool(name="small", bufs=2) as small, \
             tc.tile_pool(name="dram", bufs=1, space="DRAM") as dram:

            # ---- constants ----
            ident = big.tile([P, P], BF16)
            make_identity(nc, ident[:])
            # stacked double identity [I64; I64]: lhsT.T @ dident fuses the
            # R^T transpose with the hi+lo plane sum in a single PE op
            dident = big.tile([P, E], BF16)
            make_identity(nc, dident[0:E, 0:E])
            make_identity(nc, dident[E:2 * E, 0:E])
            # touch Exp once so the ACT table load is off the softmax path
            warm = small.tile([1, 1], F32, tag="warm")
            nc.gpsimd.memset(warm[:], 0.0)
            nc.scalar.activation(
                warm[:], warm[:], mybir.ActivationFunctionType.Exp
            )
            LSHIFT = 80.0
            nshift = small.tile([P, 1], F32, tag="nshift")
            nc.gpsimd.memset(nshift[:], -LSHIFT)

            # ---- input DMAs, all issued up front on the SP queue ----
            wqkt_sb = big.tile([P, DT, 2 * E], BF16)
            nc.sync.dma_start(out=wqkt_sb[:], in_=wqkt_v)

            xt_q = []
            for j in range(8):
                xq = big.tile([P, DT, 512], BF16, tag=f"xt_q{j}")
                nc.sync.dma_start(
                    out=xq[:], in_=xt_v[:, :, j * 512:(j + 1) * 512]
                )
                xt_q.append(xq)

            wvt_sb = big.tile([P, DT, D], BF16, tag="wvt")
            nc.sync.dma_start(out=wvt_sb[:], in_=wvt_v)

            # n-block-major adjacency stream: block j = all 32 m-tiles x
            # cols j*512..(j+1)*512, in 4 sub-DMAs of 8 m-tiles each. Each
            # block's logits complete while later blocks still stream.
            at_tiles = {}
            for j in range(4):
                for s in range(4):
                    t = atp.tile([P, 8, 512], F8, tag="at_t",
                                 name=f"at{j}_{s}")
                    nc.sync.dma_start(
                        out=t[:],
                        in_=at_v[:, 8 * s:8 * s + 8, j * 512:(j + 1) * 512],
                    )
                    at_tiles[(j, s)] = t

            # one tile per writer: the tile framework tracks dependencies at
            # tile granularity, so shared destination tiles would falsely
            # serialize writes issued on different engines.
            qkt_c = [
                big.tile([P, 512], BF16, tag=f"qkt{j}", name=f"qkt{j}")
                for j in range(8)
            ]
            q_nat = big.tile([P, TH, E], BF16)
            # K per S3 group (4 m-tiles), packed (hi 0:E | lo E:2E) per tile
            k_t = [
                big.tile([P, 4, 2 * E], F8, tag=f"k{g}", name=f"k{g}")
                for g in range(8)
            ]
            v_ev = big.tile([P, TH // 2, D], BF16, tag="v_ev")
            v_od = big.tile([P, TH // 2, D], BF16, tag="v_od")
            rt_c = [
                big.tile([P, 512], BF16, tag=f"rt{rc}", name=f"rt{rc}")
                for rc in range(4)
            ]
            l_sb = big.tile([P, TH], F32)

            # ---- S2: fused QK^T projection, bf16 ----
            with tc.tile_pool(name="ps_a", bufs=2, space="PSUM") as ps_a, \
                 tc.tile_pool(name="ps_b", bufs=2, space="PSUM") as ps_b:
                for j in range(8):
                    psum_qk = ps_a.tile([P, 512], F32, tag="pa", name=f"pqk{j}")
                    for di in range(DT):
                        nc.tensor.matmul(
                            psum_qk[:],
                            wqkt_sb[:, di, :],
                            xt_q[j][:, di, :],
                            start=(di == 0),
                            stop=(di == DT - 1),
                        )
                    nc.scalar.copy(out=qkt_c[j][:], in_=psum_qk[:])

                # ---- S3: transposes -> Q natural + K packed fp8 hi|lo ----
                for g in range(MT // 4):
                    ptr4 = ps_b.tile([P, 512], BF16, tag="pb", name=f"tr3_{g}")
                    for i in range(4):
                        t = g * 4 + i
                        nc.tensor.transpose(
                            ptr4[:, i * P:(i + 1) * P],
                            qkt_c[t // 4][:, (t % 4) * P:(t % 4 + 1) * P],
                            ident[:],
                        )
                    ptr4_v = ptr4[:].rearrange("p (c w) -> p c w", c=4)
                    if g < TH // 4:
                        nc.scalar.copy(
                            out=q_nat[:, g * 4:(g + 1) * 4, :],
                            in_=ptr4_v[:, :, 0:E],
                        )
                    nc.vector.tensor_copy(
                        out=k_t[g][:, :, 0:E], in_=ptr4_v[:, :, E:2 * E]
                    )
                    nc.vector.tensor_tensor(
                        out=k_t[g][:, :, E:2 * E],
                        in0=ptr4_v[:, :, E:2 * E],
                        in1=k_t[g][:, :, 0:E],
                        op=mybir.AluOpType.subtract,
                    )

            # ---- V projection interleaved 1:1 with S4 DoubleRow pairs ----
            # S4: R^T = (adj @ [K_hi K_lo])^T. lhsT [128, 2, 128]: m-tile
            # pair, cols = (hi 64 | lo 64); rhs [128, 2, 512]: same pair of
            # adjacency m-tiles. One pass over adj fills PSUM rows 0:64 (hi)
            # and 64:128 (lo). V-tile t fills the PE while at-tile t+1 lands.
            with tc.tile_pool(name="ps_r", bufs=1, space="PSUM") as ps_r, \
                 tc.tile_pool(name="ps_s", bufs=2, space="PSUM") as ps_s:
                with tc.tile_pool(name="ps_v", bufs=2, space="PSUM") as ps_v:
                    # one psum tile per n-block; each block's tail (psum
                    # read, z-dot, exp partial) runs while later blocks
                    # still stream in.
                    psum_rj = [
                        ps_r.tile([P, 512], F32, tag=f"pr{j}", name=f"pr{j}")
                        for j in range(4)
                    ]
                    e_sb = small.tile([P, TH], F32, tag="e_sb")
                    s_p = [
                        small.tile([P, 1], F32, tag=f"s_p{j}", name=f"s_p{j}")
                        for j in range(4)
                    ]
                    s_acc = [
                        small.tile([P, 1], F32, tag=f"s_a{j}", name=f"s_a{j}")
                        for j in range(3)
                    ]
                    def v_tile(t):
                        psum_v = ps_v.tile([P, 512], F32, tag="pv",
                                           name=f"psv{t}")
                        xr = xt_q[t // 4]
                        xo = (t % 4) * P
                        for di in range(DT):
                            nc.tensor.matmul(
                                psum_v[:],
                                xr[:, di, xo:xo + P],
                                wvt_sb[:, di, :],
                                start=(di == 0),
                                stop=(di == DT - 1),
                            )
                        if t % 2 == 0:
                            nc.vector.tensor_copy(
                                out=v_ev[:, t // 2, :], in_=psum_v[:]
                            )
                        else:
                            nc.scalar.copy(out=v_od[:, t // 2, :], in_=psum_v[:])

                    for j in range(4):
                        for s in range(4):
                            # front-load V: 2 tiles per sub in the first 8
                            # subs, so DVE/ACT are clear for the later
                            # blocks' logit tails
                            si = 4 * j + s
                            if si < 6:
                                v_tile(2 * si)
                                v_tile(2 * si + 1)
                            elif si < 10:
                                v_tile(6 + si)

                            a_t = at_tiles[(j, s)]
                            for gi in range(4):
                                g = 4 * s + gi
                                kt = k_t[g // 2][:, 2 * (g % 2):2 * (g % 2) + 2, :]
                                nc.tensor.matmul(
                                    psum_rj[j][:],
                                    kt,
                                    a_t[:, 2 * gi:2 * gi + 2, :],
                                    start=(s == 0 and gi == 0),
                                    stop=(s == 3 and gi == 3),
                                    perf_mode=DR,
                                    skip_group_check=True,
                                )

                        # ---- block-j tail: psum -> SBUF, z = hi+lo via
                        # double-identity matmul, l = q.z, exp partial ----
                        # the last block's copy goes to DVE: ACT is still
                        # finishing block 2's exp when block 3's psum lands
                        cpy = (nc.vector.tensor_copy if j % 2 == 0
                               else nc.scalar.copy)
                        cpy(out=rt_c[j][:], in_=psum_rj[j][:])
                        zp4 = ps_s.tile([P, 4, E], F32, tag="ps",
                                        name=f"z5_{j}")
                        for i in range(4):
                            nc.tensor.matmul(
                                zp4[:, i, :],
                                rt_c[j][:, i * P:(i + 1) * P],
                                dident[:],
                                start=True,
                                stop=True,
                            )
                        z4 = small.tile([P, 4, E], BF16, tag="z4",
                                        name=f"z4_{j}")
                        nc.vector.tensor_tensor(
                            out=z4[:], in0=zp4[:],
                            in1=q_nat[:, j * 4:(j + 1) * 4, :],
                            op=mybir.AluOpType.mult,
                        )
                        nc.vector.tensor_reduce(
                            out=l_sb[:, j * 4:(j + 1) * 4], in_=z4[:],
                            axis=mybir.AxisListType.X, op=mybir.AluOpType.add,
                        )
                        nc.scalar.activation(
                            e_sb[:, j * 4:(j + 1) * 4],
                            l_sb[:, j * 4:(j + 1) * 4],
                            mybir.ActivationFunctionType.Exp,
                            bias=nshift[:, 0:1], scale=1.0,
                            accum_out=s_p[j][:],
                        )
                        if j == 1:
                            nc.vector.tensor_tensor(
                                out=s_acc[0][:], in0=s_p[0][:], in1=s_p[1][:],
                                op=mybir.AluOpType.add,
                            )
                        elif j == 2:
                            nc.vector.tensor_tensor(
                                out=s_acc[1][:], in0=s_acc[0][:],
                                in1=s_p[2][:],
                                op=mybir.AluOpType.add,
                            )
                        elif j == 3:
                            nc.vector.tensor_tensor(
                                out=s_acc[2][:], in0=s_acc[1][:],
                                in1=s_p[3][:],
                                op=mybir.AluOpType.add,
                            )
                s_loc = s_acc[2]
                s_red = small.tile([P, 1], F32, tag="s_red")
                nc.gpsimd.partition_all_reduce(
                    s_red[:], s_loc[:], channels=P,
                    reduce_op=bass_isa.ReduceOp.add,
                )

                cc_in = dram.tile([1, 1], F32)
                cc_out = dram.tile([1, 1], F32)
                nc.sync.dma_start(out=cc_in[:], in_=s_red[0:1, :])
                if single:
                    nc.sync.dma_start(out=cc_out[:], in_=cc_in[:])
                else:
                    nc.gpsimd.collective_compute(
                        "AllReduce",
                        mybir.AluOpType.add,
                        replica_groups=GROUPS,
                        ins=[cc_in[:].opt()],
                        outs=[cc_out[:].opt()],
                    )
                # broadcast-load the pair total to every partition; each
                # partition redundantly computes 1/S.
                gath_bc = small.tile([P, 1], F32, tag="gath_bc")
                nc.sync.dma_start(
                    out=gath_bc[:],
                    in_=cc_out[:].rearrange("a b -> (a b)").unsqueeze(0)
                    .broadcast_to((P, 1)),
                )
                s_inv = small.tile([P, 1], F32, tag="s_inv")
                nc.vector.reciprocal(s_inv[:], gath_bc[:])

                # w = exp(l - LSHIFT) / S
                w_sb = small.tile([P, TH], F32, tag="w_sb")
                nc.vector.tensor_scalar_mul(w_sb[:], e_sb[:], s_inv[:, 0:1])

                # ---- S7: scale V by w and store ----
                OTC = 2
                o_t = None
                for t in range(TH):
                    if t % OTC == 0:
                        o_t = ostream.tile(
                            [P, OTC, D], F32, tag="o_t", name=f"ot{t}"
                        )
                    if t % 2 == 0:
                        nc.vector.tensor_scalar_mul(
                            o_t[:, t % OTC, :], v_ev[:, t // 2, :],
                            w_sb[:, t:t + 1],
                        )
                    else:
                        nc.scalar.mul(
                            out=o_t[:, t % OTC, :], in_=v_od[:, t // 2, :],
                            mul=w_sb[:, t:t + 1],
                        )
                    if t % OTC == OTC - 1:
                        g0 = t - (OTC - 1)
                        nc.sync.dma_start(
                            out=out_v[:, g0:g0 + OTC, :], in_=o_t[:]
                        )

    nc.compile()
    return nc


def prepare_in_maps(x, adj_matrix, W_qk, W_v):
    import ml_dtypes

    x = np.asarray(x, dtype=np.float32)
    adj = np.asarray(adj_matrix, dtype=np.float32)
    wqkt = np.ascontiguousarray(np.asarray(W_qk, dtype=np.float32).T)
    wqkt = wqkt.copy()
    wqkt[:, :E] *= 1.0 / np.sqrt(E)          # fold attention scale into W_q
    wqkt = wqkt.astype(ml_dtypes.bfloat16)
    wvt = np.ascontiguousarray(
        np.asarray(W_v, dtype=np.float32).T
    ).astype(ml_dtypes.bfloat16)

    in_maps = []
    for c in range(NCORES):
        b, h = divmod(c, 2)
        xt_b = x[b].T                                    # (D, N)
        if h == 1:
            xt_c = np.concatenate([xt_b[:, NH:], xt_b[:, :NH]], axis=1)
        else:
            xt_c = xt_b
        xt_c = np.ascontiguousarray(xt_c).astype(ml_dtypes.bfloat16)
        at_b = adj[b].T[:, h * NH:(h + 1) * NH]          # (N m-rows, NH cols)
        if h == 1:
            at_c = np.concatenate([at_b[NH:], at_b[:NH]], axis=0)
        else:
            at_c = at_b
        at_c = np.ascontiguousarray(at_c).astype(ml_dtypes.float8_e4m3)
        in_maps.append({"xt": xt_c, "at": at_c, "wqkt": wqkt, "wvt": wvt})
    return in_maps


def kernel(x, adj_matrix, W_qk, W_v):
    in_maps = prepare_in_maps(x, adj_matrix, W_qk, W_v)
    nc = _build()
    import os

    trace = os.environ.get("CAMIL_TRACE") == "1"
    kwargs = {}
    if trace:
        kwargs = {"trace": True, "trace_cores": list(range(NCORES))}
    res = run_bass_kernel_spmd(nc, in_maps, core_ids=list(range(NCORES)), **kwargs)

    global LAST_EXEC_NS, LAST_TRACE
    LAST_EXEC_NS = res.exec_time_ns
    LAST_TRACE = res.instructions_and_trace[1] if res.instructions_and_trace else None

    out = np.empty((B, N, D), dtype=np.float32)
    for c in range(NCORES):
        b, h = divmod(c, 2)
        out[b, h * NH:(h + 1) * NH] = res.results[c]["out"]
    return out


LAST_EXEC_NS = None
LAST_TRACE = None


# revision 54
# speedup vs baseline: 1.0002x; 1.0002x over previous
"""CAMIL self-attention kernel for 8 Trainium2 NeuronCores.

Reference computation (per bag b of B=4, N=4096 instances, D=512 features):
    qk = x @ W_qk.T ; q, k = split(qk)          (att dim E=64)
    v  = x @ W_v.T
    logits_n = (1/8) * sum_m adj[n,m] * (q_n . k_m)
             = (q_n/8) . (adj @ k)_n
    w = softmax(logits over N) ; out = w * v

Sharding: 2 cores per bag, each core owns NH=2048 rows (query dim). Inputs
are pre-rotated on host so every core's rows are local tiles 0..15; the
softmax normalization is completed with a tiny AllReduce of the local
sumexp between the two cores of each bag.

Numerics: x/W in bf16 (PE multiplies at fp22, accumulates fp32), adj exact
in fp8 e4m3, K split into fp8 hi+lo planes packed side-by-side in the
DoubleRow stationary operand so a single pass over the adjacency computes
both planes into PSUM partitions 0:64 (hi) and 64:128 (lo) at the fp8
DoubleRow rate. V-projection matmuls are interleaved 1:1 with the adj@K
pairs so the PE fills the adjacency-stream gaps. Logit dot + softmax in
fp32 on DVE/ACT; a fixed shift exp(l - 80) keeps exp in fp32 range
(logits ~N(0,20), per-bag max ~75-100) and removes the serial
max-reduction from the critical path.
"""

import sys

sys.path.insert(0, "/opt/trn_rl_repo")

import numpy as np

import concourse.bass as bass
import concourse.tile as tile
from concourse import bacc, bass_isa, mybir
from concourse.bass_utils import run_bass_kernel_spmd
from concourse.masks import make_identity

B, N, D, E = 4, 4096, 512, 64
P = 128
NCORES = 8
NH = N // 2        # rows per core
MT = N // P        # 32 m-tiles per bag
MP = MT // 2       # 16 m-tile pairs (DoubleRow covers 2 k-tiles/inst)
TH = NH // P       # 16 row-tiles per core
DT = D // P        # 4 d-tiles
F32 = mybir.dt.float32
BF16 = mybir.dt.bfloat16
F8 = mybir.dt.float8e4
DR = mybir.MatmulPerfMode.DoubleRow
GROUPS = [[0, 1], [2, 3], [4, 5], [6, 7]]


def _build(single=False):
    # single=True: replace the cross-core AllReduce with a local DMA so the
    # module has no collectives (for TimelineSim cost modeling only).
    nc = bacc.Bacc("TRN2", target_bir_lowering=False, num_devices=NCORES)

    xt = nc.dram_tensor("xt", [D, N], BF16, kind="ExternalInput")
    at = nc.dram_tensor("at", [N, NH], F8, kind="ExternalInput")
    wqkt = nc.dram_tensor("wqkt", [D, 2 * E], BF16, kind="ExternalInput")
    wvt = nc.dram_tensor("wvt", [D, D], BF16, kind="ExternalInput")
    out = nc.dram_tensor("out", [NH, D], F32, kind="ExternalOutput")

    xt_v = xt.ap().rearrange("(o p) n -> p o n", p=P)        # [128, 4, 4096]
    at_v = at.ap().rearrange("(mo p) n -> p mo n", p=P)      # [128, 32, 2048]
    wqkt_v = wqkt.ap().rearrange("(o p) e -> p o e", p=P)    # [128, 4, 128]
    wvt_v = wvt.ap().rearrange("(o p) e -> p o e", p=P)      # [128, 4, 512]
    out_v = out.ap().rearrange("(t p) e -> p t e", p=P)      # [128, 16, 512]

    with tile.TileContext(nc) as tc:
        with tc.tile_pool(name="big", bufs=1) as big, \
             tc.tile_pool(name="atp", bufs=16) as atp, \
             tc.tile_pool(name="ostream", bufs=4) as ostream, \
             tc.tile_pool(name="small", bufs=2) as small, \
             tc.tile_pool(name="dram", bufs=1, space="DRAM") as dram:

            # ---- constants ----
            ident = big.tile([P, P], BF16)
            make_identity(nc, ident[:])
            # stacked double identity [I64; I64]: lhsT.T @ dident fuses the
            # R^T transpose with the hi+lo plane sum in a single PE op
            dident = big.tile([P, E], BF16)
            make_identity(nc, dident[0:E, 0:E])
            make_identity(nc, dident[E:2 * E, 0:E])
            # touch Exp once so the ACT table load is off the softmax path
            warm = small.tile([1, 1], F32, tag="warm")
            nc.gpsimd.memset(warm[:], 0.0)
            nc.scalar.activation(
                warm[:], warm[:], mybir.ActivationFunctionType.Exp
            )
            LSHIFT = 80.0
            nshift = small.tile([P, 1], F32, tag="nshift")
            nc.gpsimd.memset(nshift[:], -LSHIFT)

            # ---- input DMAs, all issued up front on the SP queue ----
            wqkt_sb = big.tile([P, DT, 2 * E], BF16)
            nc.sync.dma_start(out=wqkt_sb[:], in_=wqkt_v)

            xt_q = []
            for j in range(8):
                xq = big.tile([P, DT, 512], BF16, tag=f"xt_q{j}")
                nc.sync.dma_start(
                    out=xq[:], in_=xt_v[:, :, j * 512:(j + 1) * 512]
                )
                xt_q.append(xq)

            wvt_sb = big.tile([P, DT, D], BF16, tag="wvt")
            nc.sync.dma_start(out=wvt_sb[:], in_=wvt_v)

            # n-block-major adjacency stream: block j = all 32 m-tiles x
            # cols j*512..(j+1)*512, in 4 sub-DMAs of 8 m-tiles each. Each
            # block's logits complete while later blocks still stream.
            at_tiles = {}
            for j in range(4):
                for s in range(4):
                    t = atp.tile([P, 8, 512], F8, tag="at_t",
                                 name=f"at{j}_{s}")
                    nc.sync.dma_start(
                        out=t[:],
                        in_=at_v[:, 8 * s:8 * s + 8, j * 512:(j + 1) * 512],
                    )
                    at_tiles[(j, s)] = t

            # one tile per writer: the tile framework tracks dependencies at
            # tile granularity, so shared destination tiles would falsely
            # serialize writes issued on different engines.
            qkt_c = [
                big.tile([P, 512], BF16, tag=f"qkt{j}", name=f"qkt{j}")
                for j in range(8)
            ]
            q_nat = big.tile([P, TH, E], BF16)
            # K per S3 group (4 m-tiles), packed (hi 0:E | lo E:2E) per tile
            k_t = [
                big.tile([P, 4, 2 * E], F8, tag=f"k{g}", name=f"k{g}")
                for g in range(8)
            ]
            v_ev = big.tile([P, TH // 2, D], BF16, tag="v_ev")
            v_od = big.tile([P, TH // 2, D], BF16, tag="v_od")
            rt_c = [
                big.tile([P, 512], BF16, tag=f"rt{rc}", name=f"rt{rc}")
                for rc in range(4)
            ]
            l_sb = big.tile([P, TH], F32)

            # ---- S2: fused QK^T projection, bf16 ----
            with tc.tile_pool(name="ps_a", bufs=2, space="PSUM") as ps_a, \
                 tc.tile_pool(name="ps_b", bufs=2, space="PSUM") as ps_b:
                for j in range(8):
                    psum_qk = ps_a.tile([P, 512], F32, tag="pa", name=f"pqk{j}")
                    for di in range(DT):
                        nc.tensor.matmul(
                            psum_qk[:],
                            wqkt_sb[:, di, :],
                            xt_q[j][:, di, :],
                            start=(di == 0),
                            stop=(di == DT - 1),
                        )
                    nc.scalar.copy(out=qkt_c[j][:], in_=psum_qk[:])

                # ---- S3: transposes -> Q natural + K packed fp8 hi|lo ----
                for g in range(MT // 4):
                    ptr4 = ps_b.tile([P, 512], BF16, tag="pb", name=f"tr3_{g}")
                    for i in range(4):
                        t = g * 4 + i
                        nc.tensor.transpose(
                            ptr4[:, i * P:(i + 1) * P],
                            qkt_c[t // 4][:, (t % 4) * P:(t % 4 + 1) * P],
                            ident[:],
                        )
                    ptr4_v = ptr4[:].rearrange("p (c w) -> p c w", c=4)
                    if g < TH // 4:
                        nc.scalar.copy(
                            out=q_nat[:, g * 4:(g + 1) * 4, :],
                            in_=ptr4_v[:, :, 0:E],
                        )
                    nc.vector.tensor_copy(
                        out=k_t[g][:, :, 0:E], in_=ptr4_v[:, :, E:2 * E]
                    )
                    nc.vector.tensor_tensor(
                        out=k_t[g][:, :, E:2 * E],
                        in0=ptr4_v[:, :, E:2 * E],
                        in1=k_t[g][:, :, 0:E],
                        op=mybir.AluOpType.subtract,
                    )

            # ---- V projection interleaved 1:1 with S4 DoubleRow pairs ----
            # S4: R^T = (adj @ [K_hi K_lo])^T. lhsT [128, 2, 128]: m-tile
            # pair, cols = (hi 64 | lo 64); rhs [128, 2, 512]: same pair of
            # adjacency m-tiles. One pass over adj fills PSUM rows 0:64 (hi)
            # and 64:128 (lo). V-tile t fills the PE while at-tile t+1 lands.
            with tc.tile_pool(name="ps_r", bufs=1, space="PSUM") as ps_r, \
                 tc.tile_pool(name="ps_s", bufs=2, space="PSUM") as ps_s:
                with tc.tile_pool(name="ps_v", bufs=2, space="PSUM") as ps_v:
                    # one psum tile per n-block; each block's tail (psum
                    # read, z-dot, exp partial) runs while later blocks
                    # still stream in.
                    psum_rj = [
                        ps_r.tile([P, 512], F32, tag=f"pr{j}", name=f"pr{j}")
                        for j in range(4)
                    ]
                    e_sb = small.tile([P, TH], F32, tag="e_sb")
                    s_p = [
                        small.tile([P, 1], F32, tag=f"s_p{j}", name=f"s_p{j}")
                        for j in range(4)
                    ]
                    s_acc = [
                        small.tile([P, 1], F32, tag=f"s_a{j}", name=f"s_a{j}")
                        for j in range(3)
                    ]
                    def v_tile(t):
                        psum_v = ps_v.tile([P, 512], F32, tag="pv",
                                           name=f"psv{t}")
                        xr = xt_q[t // 4]
                        xo = (t % 4) * P
                        for di in range(DT):
                            nc.tensor.matmul(
                                psum_v[:],
                                xr[:, di, xo:xo + P],
                                wvt_sb[:, di, :],
                                start=(di == 0),
                                stop=(di == DT - 1),
                            )
                        if t % 2 == 0:
                            nc.vector.tensor_copy(
                                out=v_ev[:, t // 2, :], in_=psum_v[:]
                            )
                        else:
                            nc.scalar.copy(out=v_od[:, t // 2, :], in_=psum_v[:])

                    for j in range(4):
                        for s in range(4):
                            # front-load V: 2 tiles per sub in the first 8
                            # subs, so DVE/ACT are clear for the later
                            # blocks' logit tails
                            si = 4 * j + s
                            if si < 6:
                                v_tile(2 * si)
                                v_tile(2 * si + 1)
                            elif si < 10:
                                v_tile(6 + si)

                            a_t = at_tiles[(j, s)]
                            for gi in range(4):
                                g = 4 * s + gi
                                kt = k_t[g // 2][:, 2 * (g % 2):2 * (g % 2) + 2, :]
                                nc.tensor.matmul(
                                    psum_rj[j][:],
                                    kt,
                                    a_t[:, 2 * gi:2 * gi + 2, :],
                                    start=(s == 0 and gi == 0),
                                    stop=(s == 3 and gi == 3),
                                    perf_mode=DR,
                                    skip_group_check=True,
                                )

                        # ---- block-j tail: psum -> SBUF, z = hi+lo via
                        # double-identity matmul, l = q.z, exp partial ----
                        # the last block's copy goes to DVE: ACT is still
                        # finishing block 2's exp when block 3's psum lands
                        cpy = (nc.vector.tensor_copy if j != 1
                               else nc.scalar.copy)
                        cpy(out=rt_c[j][:], in_=psum_rj[j][:])
                        zp4 = ps_s.tile([P, 4, E], F32, tag="ps",
                                        name=f"z5_{j}")
                        for i in range(4):
                            nc.tensor.matmul(
                                zp4[:, i, :],
                                rt_c[j][:, i * P:(i + 1) * P],
                                dident[:],
                                start=True,
                                stop=True,
                            )
                        z4 = small.tile([P, 4, E], BF16, tag="z4",
                                        name=f"z4_{j}")
                        nc.vector.tensor_tensor(
                            out=z4[:], in0=zp4[:],
                            in1=q_nat[:, j * 4:(j + 1) * 4, :],
                            op=mybir.AluOpType.mult,
                        )
                        nc.vector.tensor_reduce(
                            out=l_sb[:, j * 4:(j + 1) * 4], in_=z4[:],
                            axis=mybir.AxisListType.X, op=mybir.AluOpType.add,
                        )
                        nc.scalar.activation(
                            e_sb[:, j * 4:(j + 1) * 4],
                            l_sb[:, j * 4:(j + 1) * 4],
                            mybir.ActivationFunctionType.Exp,
                            bias=nshift[:, 0:1], scale=1.0,
                            accum_out=s_p[j][:],
                        )
                        if j == 1:
                            nc.vector.tensor_tensor(
                                out=s_acc[0][:], in0=s_p[0][:], in1=s_p[1][:],
                                op=mybir.AluOpType.add,
                            )
                        elif j == 2:
                            nc.vector.tensor_tensor(
                                out=s_acc[1][:], in0=s_acc[0][:],
                                in1=s_p[2][:],
                                op=mybir.AluOpType.add,
                            )
                        elif j == 3:
                            nc.vector.tensor_tensor(
                                out=s_acc[2][:], in0=s_acc[1][:],
                                in1=s_p[3][:],
                                op=mybir.AluOpType.add,
                            )
                s_loc = s_acc[2]
                s_red = small.tile([P, 1], F32, tag="s_red")
                nc.gpsimd.partition_all_reduce(
                    s_red[:], s_loc[:], channels=P,
                    reduce_op=bass_isa.ReduceOp.add,
                )

                cc_in = dram.tile([1, 1], F32)
                cc_out = dram.tile([1, 1], F32)
                nc.sync.dma_start(out=cc_in[:], in_=s_red[0:1, :])
                if single:
                    nc.sync.dma_start(out=cc_out[:], in_=cc_in[:])
                else:
                    nc.gpsimd.collective_compute(
                        "AllReduce",
                        mybir.AluOpType.add,
                        replica_groups=GROUPS,
                        ins=[cc_in[:].opt()],
                        outs=[cc_out[:].opt()],
                    )
                # broadcast-load the pair total to every partition; each
                # partition redundantly computes 1/S.
                gath_bc = small.tile([P, 1], F32, tag="gath_bc")
                nc.sync.dma_start(
                    out=gath_bc[:],
                    in_=cc_out[:].rearrange("a b -> (a b)").unsqueeze(0)
                    .broadcast_to((P, 1)),
                )
                s_inv = small.tile([P, 1], F32, tag="s_inv")
                nc.vector.reciprocal(s_inv[:], gath_bc[:])

                # w = exp(l - LSHIFT) / S
                w_sb = small.tile([P, TH], F32, tag="w_sb")
                nc.vector.tensor_scalar_mul(w_sb[:], e_sb[:], s_inv[:, 0:1])

                # ---- S7: scale V by w and store ----
                OTC = 2
                o_t = None
                for t in range(TH):
                    if t % OTC == 0:
                        o_t = ostream.tile(
                            [P, OTC, D], F32, tag="o_t", name=f"ot{t}"
                        )
                    if t % 2 == 0:
                        nc.vector.tensor_scalar_mul(
                            o_t[:, t % OTC, :], v_ev[:, t // 2, :],
                            w_sb[:, t:t + 1],
                        )
                    else:
                        nc.scalar.mul(
                            out=o_t[:, t % OTC, :], in_=v_od[:, t // 2, :],
                            mul=w_sb[:, t:t + 1],
                        )
                    if t % OTC == OTC - 1:
                        g0 = t - (OTC - 1)
                        nc.sync.dma_start(
                            out=out_v[:, g0:g0 + OTC, :], in_=o_t[:]
                        )

    nc.compile()
    return nc


def prepare_in_maps(x, adj_matrix, W_qk, W_v):
    import ml_dtypes

    x = np.asarray(x, dtype=np.float32)
    adj = np.asarray(adj_matrix, dtype=np.float32)
    wqkt = np.ascontiguousarray(np.asarray(W_qk, dtype=np.float32).T)
    wqkt = wqkt.copy()
    wqkt[:, :E] *= 1.0 / np.sqrt(E)          # fold attention scale into W_q
    wqkt = wqkt.astype(ml_dtypes.bfloat16)
    wvt = np.ascontiguousarray(
        np.asarray(W_v, dtype=np.float32).T
    ).astype(ml_dtypes.bfloat16)

    in_maps = []
    for c in range(NCORES):
        b, h = divmod(c, 2)
        xt_b = x[b].T                                    # (D, N)
        if h == 1:
            xt_c = np.concatenate([xt_b[:, NH:], xt_b[:, :NH]], axis=1)
        else:
            xt_c = xt_b
        xt_c = np.ascontiguousarray(xt_c).astype(ml_dtypes.bfloat16)
        at_b = adj[b].T[:, h * NH:(h + 1) * NH]          # (N m-rows, NH cols)
        if h == 1:
            at_c = np.concatenate([at_b[NH:], at_b[:NH]], axis=0)
        else:
            at_c = at_b
        at_c = np.ascontiguousarray(at_c).astype(ml_dtypes.float8_e4m3)
        in_maps.append({"xt": xt_c, "at": at_c, "wqkt": wqkt, "wvt": wvt})
    return in_maps


def kernel(x, adj_matrix, W_qk, W_v):
    in_maps = prepare_in_maps(x, adj_matrix, W_qk, W_v)
    nc = _build()
    import os

    trace = os.environ.get("CAMIL_TRACE") == "1"
    kwargs = {}
    if trace:
        kwargs = {"trace": True, "trace_cores": list(range(NCORES))}
    res = run_bass_kernel_spmd(nc, in_maps, core_ids=list(range(NCORES)), **kwargs)

    global LAST_EXEC_NS, LAST_TRACE
    LAST_EXEC_NS = res.exec_time_ns
    LAST_TRACE = res.instructions_and_trace[1] if res.instructions_and_trace else None

    out = np.empty((B, N, D), dtype=np.float32)
    for c in range(NCORES):
        b, h = divmod(c, 2)
        out[b, h * NH:(h + 1) * NH] = res.results[c]["out"]
    return out


LAST_EXEC_NS = None
LAST_TRACE = None
